# revision 42
# baseline (speedup 1.0000x reference)
import sys

sys.path.insert(0, "/opt/trn_rl_repo")

from contextlib import ExitStack

import ml_dtypes
import numpy as np

import concourse.bass as bass
import concourse.mybir as mybir
import concourse.tile as tile
from concourse import bacc, bass_utils

N, OBS, ENC, ACT, K = 16384, 512, 512, 64, 8
ALPHA = 1.0
NCORES = 8
P = 128
F32 = mybir.dt.float32
FP8 = mybir.dt.float8e4
NP_FP8 = ml_dtypes.float8_e4m3
AX = mybir.AluOpType
DR = mybir.MatmulPerfMode.DoubleRow

# Routed MoE loss, one fp8 DoubleRow kernel per core (SPMD). Per routed row n
# with expert k:  diff = x0 @ (W^T A_k^T) + u @ B_k - x1 @ W^T ;
# loss += ||diff||^2.  Contraction z = [x0(512); u(64)+pad(64); x1(512);
# zeros(128)] = 10 chunks of 128 processed as 5 DoubleRow matmuls (2 chunks
# each) per 128-row tile into one PSUM bank. Weights are x16 scaled into the
# fp8 e4m3 sweet spot; the host routes (f64) and gathers rows.
#
# Load balance: every core runs NSEG fixed-size segments (e.g. [5,4,4,4]
# tiles); a host bin-packer assigns each (core, segment) to one expert, whose
# weight matrix lands in that segment's SBUF slot. All cores run the same
# instruction stream; total work is ceil(total_tiles/8) tiles per core
# instead of max_k tiles_k.
#
# DMA layouts are per-partition contiguous (the engines are descriptor-rate
# limited at ~45ns/packet, so packets are 1.5-5KB); pieces are issued in
# consumption order, mostly on one queue so earlier-needed pieces keep
# priority. Zero x zero warmup matmuls climb the PE DVFS ramp during the
# fill. Squares run on ACT with a few pairs on DVE (2-pass: DVE cannot
# dual-read PSUM) placed so neither engine gates a segment boundary.
NCH = 10  # contraction chunks incl. u-pad and trailing zero pad
ZCH = 9  # chunks with host-provided data (zero pad chunk is memset)
WSCALE = 16.0
NSEG = 4
NWARM = 8
SWI = True  # DoubleRowSwInterleave: host pre-interleaves stationary z pairs
NPAIR = NCH // 2


def _seg_sizes(T):
    base, rem = divmod(T, NSEG)
    return tuple([base + 1] * rem + [base] * (NSEG - rem))


def _groups(sz, is_last):
    # psum-group tiling of a segment: pairs, plus (last segment, even size)
    # two trailing singles so the final square is a cheap 512-wide ACT op
    if is_last and sz >= 4 and sz % 2 == 0:
        g = [(i, i + 1) for i in range(0, sz - 2, 2)]
        return g + [(sz - 2,), (sz - 1,)]
    return [tuple(range(i, min(i + 2, sz))) for i in range(0, sz, 2)]


def build_nc(sizes):
    nc = bacc.Bacc("TRN2", target_bir_lowering=False)
    zs, ws = [], []
    for s, sz in enumerate(sizes):
        zshape = [P, NPAIR, sz, 2, P] if SWI else [P, ZCH, sz * P]
        zs.append(nc.declare_dram_parameter(f"z{s}", zshape, FP8, isOutput=False))
        ws.append(
            nc.declare_dram_parameter(f"w{s}", [P, NCH, ENC], FP8, isOutput=False)
        )
    seg_groups = [_groups(sz, s == len(sizes) - 1) for s, sz in enumerate(sizes)]
    npairs = sum(len(g) for g in seg_groups)
    loss = nc.declare_dram_parameter("loss_out", [P, npairs], F32, isOutput=True)

    with tile.TileContext(nc) as tc, ExitStack() as ctx:
        const = ctx.enter_context(tc.tile_pool(name="const", bufs=1))
        dwork = ctx.enter_context(tc.tile_pool(name="dwork", bufs=4))
        psumA = ctx.enter_context(tc.tile_pool(name="psumA", bufs=4, space="PSUM"))

        w_sb = [
            const.tile([P, NCH, ENC], FP8, name=f"wsb{s}") for s in range(len(sizes))
        ]
        z_sb = [
            const.tile(
                [P, NPAIR, sz, 2, P] if SWI else [P, NCH, sz * P],
                FP8,
                name=f"zsb{s}",
            )
            for s, sz in enumerate(sizes)
        ]
        acc = const.tile([P, npairs], F32, name="accsb")

        warmz = const.tile([P, 5 * P], FP8, name="warmz")
        nc.gpsimd.memset(warmz[:], 0.0)
        for wt in w_sb:
            nc.gpsimd.memset(wt[:, ZCH : ZCH + 1, :], 0.0)
        if not SWI:
            for zt in z_sb:
                nc.gpsimd.memset(zt[:, ZCH : ZCH + 1, :], 0.0)

        def zslice(t, lo, hi):
            # piece covering chunk range [2lo, 2hi) == pair range [lo, hi)
            return t[:, lo:hi, :, :, :] if SWI else t[:, 2 * lo : min(2 * hi, ZCH), :]
        # 14 pieces over two queues: the ~0.6us serial issue cost per DMA is
        # the scarce resource, so pieces are consolidated and ordered so
        # every sweep's data lands >=1us before the PE reaches it
        nc.sync.dma_start(w_sb[0][:, 0:2, :], ws[0][:, 0:2, :])
        nc.scalar.dma_start(zslice(z_sb[0], 0, 1), zslice(zs[0], 0, 1))
        nc.sync.dma_start(w_sb[0][:, 2:6, :], ws[0][:, 2:6, :])
        nc.scalar.dma_start(zslice(z_sb[0], 1, 3), zslice(zs[0], 1, 3))
        nc.sync.dma_start(w_sb[0][:, 6:ZCH, :], ws[0][:, 6:ZCH, :])
        nc.sync.dma_start(zslice(z_sb[0], 3, NPAIR), zslice(zs[0], 3, NPAIR))
        if len(sizes) > 1:
            nc.scalar.dma_start(w_sb[1][:, 0:2, :], ws[1][:, 0:2, :])
            nc.scalar.dma_start(zslice(z_sb[1], 0, 2), zslice(zs[1], 0, 2))
            nc.sync.dma_start(w_sb[1][:, 2:ZCH, :], ws[1][:, 2:ZCH, :])
            nc.sync.dma_start(zslice(z_sb[1], 2, NPAIR), zslice(zs[1], 2, NPAIR))
        for s in range(2, len(sizes)):
            nc.sync.dma_start(w_sb[s][:, 0:ZCH, :], ws[s][:, 0:ZCH, :])
            nc.sync.dma_start(zslice(z_sb[s], 0, NPAIR), zslice(zs[s], 0, NPAIR))

        # zero x zero warmup matmuls: climb the PE DVFS ramp during the DMA
        # fill (depend only on the memset pad chunk; result never read)
        warm = psumA.tile([P, 2, ENC], F32, name="pA")
        for wi in range(NWARM):
            nc.tensor.matmul(
                warm[:, wi % 2, :],
                warmz[:, 0:P],
                warmz[:, P : P + ENC],
                start=True,
                stop=True,
            )

        # per segment: sweep chunk-pairs across its tiles (the j-th sweep
        # only needs the j-th z/w pieces), then square+accumulate
        pbase = 0
        for s, sz in enumerate(sizes):
            groups = seg_groups[s]
            pds = [psumA.tile([P, 2, ENC], F32, name="pA") for _ in groups]
            pd_of = {}
            for g, grp in enumerate(groups):
                for ii, i in enumerate(grp):
                    pd_of[i] = (g, ii)
            if s == 0:
                order = [(j, i) for j in range(NCH // 2) for i in range(sz)]
            elif s == len(sizes) - 1:
                # last segment: its data landed long ago, so go group-major —
                # early groups' squares overlap the later groups' matmuls and
                # the kernel ends on a cheap single-tile square
                order = [
                    (j, i)
                    for grp in groups
                    for j in range(NCH // 2)
                    for i in grp
                ]
            else:
                # front-load sweeps 0-1 of the first psum pair's tiles: the
                # buffer behind pair 1 may still be read by the previous
                # segment's square for ~1us after its last matmul
                lead = list(groups[0])
                rest = [i for i in range(sz) if i not in groups[0]]
                order = (
                    [(j, i) for j in (0, 1) for i in lead]
                    + [(j, i) for j in (0, 1) for i in rest]
                    + [(j, i) for j in range(2, NCH // 2) for i in range(sz)]
                )
            for j, i in order:
                nts = slice(i * P, (i + 1) * P)
                g, ii = pd_of[i]
                lhsT = (
                    z_sb[s][:, j, i, :, :]
                    if SWI
                    else z_sb[s][:, 2 * j : 2 * j + 2, nts]
                )
                nc.tensor.matmul(
                    pds[g][:, ii, :],
                    lhsT,
                    w_sb[s][:, 2 * j : 2 * j + 2, :],
                    start=(j == 0),
                    stop=(j == NCH // 2 - 1),
                    perf_mode=mybir.MatmulPerfMode.DoubleRowSwInterleave if SWI else DR,
                )
            for i, pd in enumerate(pds):
                width = len(groups[i])
                pi = pbase + i
                if 0 < s < len(sizes) - 1 and i == 0 and width == 2:
                    sv = dwork.tile([P, 2, ENC], F32, name="sv")
                    nc.vector.tensor_scalar(sv[:], pd[:], 1.0, None, op0=AX.mult)
                    sj = dwork.tile([P, 2, ENC], F32, name="sj")
                    nc.vector.scalar_tensor_tensor(
                        sj[:],
                        sv[:],
                        1.0,
                        sv[:],
                        op0=AX.mult,
                        op1=AX.mult,
                        accum_out=acc[:, pi : pi + 1],
                    )
                else:
                    sj = dwork.tile([P, width, ENC], F32, name="sj")
                    nc.scalar.activation(
                        sj[:],
                        pd[:, 0:width, :],
                        mybir.ActivationFunctionType.Square,
                        accum_out=acc[:, pi : pi + 1],
                    )
            pbase += len(pds)
        # flush all but the last segment's accumulators mid-phase; the final
        # piece issues from the scalar engine, which also ran the last
        # squares, so no cross-engine semaphore hop sits on the critical tail
        cut = npairs - len(seg_groups[-1])
        if cut > 0:
            nc.sync.dma_start(loss[:, 0:cut], acc[:, 0:cut])
        nc.scalar.dma_start(loss[:, cut:npairs], acc[:, cut:npairs])

    nc.finalize()
    return nc


_NC_CACHE = {}


def _get_nc(sizes=None):
    if sizes is None:
        key = next(reversed(_NC_CACHE))
    else:
        key = tuple(sizes)
        if key not in _NC_CACHE:
            _NC_CACHE[key] = build_nc(key)
    return _NC_CACHE[key]


def _route(X0, W_enc, C_w, C_b):
    # f64 router on host: argmax(X0 @ W_enc.T @ C_w.T + C_b) per row
    m = (C_w.astype(np.float64) @ W_enc.astype(np.float64)).T  # [OBS, K]
    logits = X0.astype(np.float64) @ m + C_b.astype(np.float64)
    return np.argmax(logits, axis=1)


def _pack(tiles_k):
    # Assign each (core, segment-slot) to one expert so every expert's tiles
    # fit in its segments. All cores share the same slot-size pattern so the
    # device program is SPMD-identical.
    total = sum(tiles_k)
    lo = max(1, -(-total // NCORES))
    hi = max(max(tiles_k), lo)
    for T in range(lo, hi + 1):
        sizes = _seg_sizes(T)
        slots = [
            (sz, core, si) for core in range(NCORES) for si, sz in enumerate(sizes)
        ]
        slots.sort()
        assign = {}
        ok = True
        for k in sorted(range(K), key=lambda k: -tiles_k[k]):
            need = tiles_k[k]
            while need > 0:
                if not slots:
                    ok = False
                    break
                # closest fit: smallest slot covering the need, else largest
                idx = next((i for i, s in enumerate(slots) if s[0] >= need), -1)
                sz, core, si = slots.pop(idx)
                assign[(core, si)] = k
                need -= sz
            if not ok:
                break
        if ok:
            return sizes, assign
    # unreachable: T = max(tiles_k) always fits (each expert gets one core)
    raise RuntimeError("packing failed")


def make_in_maps(X1, X0, U, W_enc, A_all, B_rest, C_w, C_b):
    inds = _route(X0, W_enc, C_w, C_b)
    tiles_k = [max(1, -(-int(c) // P)) for c in np.bincount(inds, minlength=K)]
    sizes, assign = _pack(tiles_k)

    wT = W_enc.T.astype(np.float32)  # [OBS, ENC]
    ae = wT[None] @ A_all.transpose(0, 2, 1).astype(np.float32)  # [K, OBS, ENC]
    b0 = np.eye(ENC, dtype=np.float32)[:ACT]
    ball = np.concatenate([b0[None], B_rest.astype(np.float32)], axis=0)
    wmat = np.zeros((K, NCH * P, ENC), dtype=np.float32)
    wmat[:, 0:OBS] = ae * WSCALE
    wmat[:, OBS : OBS + ACT] = ball * WSCALE
    wmat[:, OBS + P : OBS + P + OBS] = -wT[None] * WSCALE
    wq = np.asarray(wmat, dtype=NP_FP8).reshape(K, NCH, P, ENC).transpose(0, 2, 1, 3)
    wq = np.ascontiguousarray(wq)  # [K, P, NCH, ENC]
    wzero = np.zeros((P, NCH, ENC), dtype=NP_FP8)

    rows_k = [np.nonzero(inds == k)[0] for k in range(K)]
    used_k = [0] * K

    in_maps = [{} for _ in range(NCORES)]
    for core in range(NCORES):
        for si, sz in enumerate(sizes):
            cap = sz * P
            k = assign.get((core, si))
            Z = np.zeros((ZCH * P, cap), dtype=np.float32)
            if k is not None:
                rk = rows_k[k][used_k[k] : used_k[k] + cap]
                used_k[k] += len(rk)
                c = len(rk)
                Z[0:OBS, :c] = X0[rk].T
                Z[OBS : OBS + ACT, :c] = U[rk].T
                Z[OBS + P : OBS + P + OBS, :c] = X1[rk].T
            zq = np.asarray(Z, dtype=NP_FP8).reshape(ZCH, P, cap)
            if SWI:
                # stationary stream for DoubleRowSwInterleave: per (pair,
                # partition, tile) the 256-byte block is [A[m-1],B[m-1],...,
                # A[0],B[0]] for pair halves A,B (see bass_interp)
                c10 = np.concatenate(
                    [zq, np.zeros((1, P, cap), NP_FP8)], axis=0
                ).reshape(NPAIR, 2, P, sz, P)
                sflat = np.empty((NPAIR, P, sz, 2 * P), dtype=NP_FP8)
                sflat[..., 0::2] = c10[:, 0][..., ::-1]
                sflat[..., 1::2] = c10[:, 1][..., ::-1]
                in_maps[core][f"z{si}"] = np.ascontiguousarray(
                    sflat.transpose(1, 0, 2, 3)
                ).reshape(P, NPAIR, sz, 2, P)
            else:
                in_maps[core][f"z{si}"] = np.ascontiguousarray(zq.transpose(1, 0, 2))
            in_maps[core][f"w{si}"] = wq[k] if k is not None else wzero
    assert all(used_k[k] == len(rows_k[k]) for k in range(K)), "packing dropped rows"
    return in_maps, sizes


def kernel(X1, X0, U, W_enc, A_all, B_rest, C_w, C_b):
    in_maps, sizes = make_in_maps(X1, X0, U, W_enc, A_all, B_rest, C_w, C_b)
    nc = _get_nc(sizes)
    res = bass_utils.run_bass_kernel_spmd(nc, in_maps, list(range(NCORES)))
    total = sum(float(r["loss_out"].sum()) for r in res.results)
    return np.float32(ALPHA * total / (WSCALE * WSCALE * ENC * N))


# revision 43
# speedup vs baseline: 1.0189x; 1.0189x over previous
import sys

sys.path.insert(0, "/opt/trn_rl_repo")

from contextlib import ExitStack

import ml_dtypes
import numpy as np

import concourse.bass as bass
import concourse.mybir as mybir
import concourse.tile as tile
from concourse import bacc, bass_utils

N, OBS, ENC, ACT, K = 16384, 512, 512, 64, 8
ALPHA = 1.0
NCORES = 8
P = 128
F32 = mybir.dt.float32
FP8 = mybir.dt.float8e4
NP_FP8 = ml_dtypes.float8_e4m3
AX = mybir.AluOpType
DR = mybir.MatmulPerfMode.DoubleRow

# Routed MoE loss, one fp8 DoubleRow kernel per core (SPMD). Per routed row n
# with expert k:  diff = x0 @ (W^T A_k^T) + u @ B_k - x1 @ W^T ;
# loss += ||diff||^2.  Contraction z = [x0(512); u(64)+pad(64); x1(512);
# zeros(128)] = 10 chunks of 128 processed as 5 DoubleRow matmuls (2 chunks
# each) per 128-row tile into one PSUM bank. Weights are x16 scaled into the
# fp8 e4m3 sweet spot; the host routes (f64) and gathers rows.
#
# Load balance: every core runs NSEG fixed-size segments (e.g. [5,4,4,4]
# tiles); a host bin-packer assigns each (core, segment) to one expert, whose
# weight matrix lands in that segment's SBUF slot. All cores run the same
# instruction stream; total work is ceil(total_tiles/8) tiles per core
# instead of max_k tiles_k.
#
# DMA layouts are per-partition contiguous (the engines are descriptor-rate
# limited at ~45ns/packet, so packets are 1.5-5KB); pieces are issued in
# consumption order, mostly on one queue so earlier-needed pieces keep
# priority. Zero x zero warmup matmuls climb the PE DVFS ramp during the
# fill. Squares run on ACT with a few pairs on DVE (2-pass: DVE cannot
# dual-read PSUM) placed so neither engine gates a segment boundary.
NCH = 10  # contraction chunks incl. u-pad and trailing zero pad
ZCH = 9  # chunks with host-provided data (zero pad chunk is memset)
WSCALE = 16.0
NSEG = 4
NWARM = 8


def _seg_sizes(T):
    base, rem = divmod(T, NSEG)
    return tuple([base + 1] * rem + [base] * (NSEG - rem))


def _groups(sz, is_last):
    # psum-group tiling of a segment: pairs, plus (last segment, even size)
    # two trailing singles so the final square is a cheap 512-wide ACT op
    if is_last and sz >= 4 and sz % 2 == 0:
        g = [(i, i + 1) for i in range(0, sz - 2, 2)]
        return g + [(sz - 2,), (sz - 1,)]
    return [tuple(range(i, min(i + 2, sz))) for i in range(0, sz, 2)]


def build_nc(sizes):
    nc = bacc.Bacc("TRN2", target_bir_lowering=False)
    zs, ws = [], []
    for s, sz in enumerate(sizes):
        zs.append(
            nc.declare_dram_parameter(f"z{s}", [P, ZCH, sz * P], FP8, isOutput=False)
        )
        ws.append(
            nc.declare_dram_parameter(f"w{s}", [P, NCH, ENC], FP8, isOutput=False)
        )
    seg_groups = [_groups(sz, s == len(sizes) - 1) for s, sz in enumerate(sizes)]
    npairs = sum(len(g) for g in seg_groups)
    loss = nc.declare_dram_parameter("loss_out", [P, npairs], F32, isOutput=True)

    with tile.TileContext(nc) as tc, ExitStack() as ctx:
        const = ctx.enter_context(tc.tile_pool(name="const", bufs=1))
        dwork = ctx.enter_context(tc.tile_pool(name="dwork", bufs=4))
        psumA = ctx.enter_context(tc.tile_pool(name="psumA", bufs=4, space="PSUM"))

        w_sb = [
            const.tile([P, NCH, ENC], FP8, name=f"wsb{s}") for s in range(len(sizes))
        ]
        z_sb = [
            const.tile([P, NCH, sz * P], FP8, name=f"zsb{s}")
            for s, sz in enumerate(sizes)
        ]
        acc = const.tile([P, npairs], F32, name="accsb")

        nc.gpsimd.memset(z_sb[0][:, ZCH : ZCH + 1, :], 0.0)
        for wt in w_sb:
            nc.gpsimd.memset(wt[:, ZCH : ZCH + 1, :], 0.0)
        for zt in z_sb[1:]:
            nc.gpsimd.memset(zt[:, ZCH : ZCH + 1, :], 0.0)
        # 14 pieces over two queues: the ~0.6us serial issue cost per DMA is
        # the scarce resource, so pieces are consolidated and ordered so
        # every sweep's data lands >=1us before the PE reaches it
        nc.sync.dma_start(w_sb[0][:, 0:2, :], ws[0][:, 0:2, :])
        nc.scalar.dma_start(z_sb[0][:, 0:2, :], zs[0][:, 0:2, :])
        nc.sync.dma_start(w_sb[0][:, 2:6, :], ws[0][:, 2:6, :])
        nc.scalar.dma_start(z_sb[0][:, 2:5, :], zs[0][:, 2:5, :])
        nc.sync.dma_start(w_sb[0][:, 6:ZCH, :], ws[0][:, 6:ZCH, :])
        nc.sync.dma_start(z_sb[0][:, 5:ZCH, :], zs[0][:, 5:ZCH, :])
        if len(sizes) > 1:
            nc.scalar.dma_start(w_sb[1][:, 0:2, :], ws[1][:, 0:2, :])
            nc.scalar.dma_start(z_sb[1][:, 0:4, :], zs[1][:, 0:4, :])
            nc.sync.dma_start(w_sb[1][:, 2:ZCH, :], ws[1][:, 2:ZCH, :])
            nc.sync.dma_start(z_sb[1][:, 4:ZCH, :], zs[1][:, 4:ZCH, :])
        for s in range(2, len(sizes)):
            nc.sync.dma_start(w_sb[s][:, 0:ZCH, :], ws[s][:, 0:ZCH, :])
            nc.sync.dma_start(z_sb[s][:, 0:ZCH, :], zs[s][:, 0:ZCH, :])

        # zero x zero warmup matmuls: climb the PE DVFS ramp during the DMA
        # fill (depend only on the memset pad chunk; result never read)
        warm = psumA.tile([P, 2, ENC], F32, name="pA")
        for wi in range(NWARM):
            nc.tensor.matmul(
                warm[:, wi % 2, :],
                z_sb[0][:, ZCH, 0:P],
                z_sb[0][:, ZCH, 0:ENC],
                start=True,
                stop=True,
            )

        # per segment: sweep chunk-pairs across its tiles (the j-th sweep
        # only needs the j-th z/w pieces), then square+accumulate
        pbase = 0
        for s, sz in enumerate(sizes):
            groups = seg_groups[s]
            pds = [psumA.tile([P, 2, ENC], F32, name="pA") for _ in groups]
            pd_of = {}
            for g, grp in enumerate(groups):
                for ii, i in enumerate(grp):
                    pd_of[i] = (g, ii)
            if s == 0:
                order = [(j, i) for j in range(NCH // 2) for i in range(sz)]
            elif s == len(sizes) - 1:
                # last segment: its data landed long ago, so go group-major —
                # early groups' squares overlap the later groups' matmuls and
                # the kernel ends on a cheap single-tile square
                order = [
                    (j, i)
                    for grp in groups
                    for j in range(NCH // 2)
                    for i in grp
                ]
            else:
                # front-load sweeps 0-1 of the first psum pair's tiles: the
                # buffer behind pair 1 may still be read by the previous
                # segment's square for ~1us after its last matmul
                lead = list(groups[0])
                rest = [i for i in range(sz) if i not in groups[0]]
                order = (
                    [(j, i) for j in (0, 1) for i in lead]
                    + [(j, i) for j in (0, 1) for i in rest]
                    + [(j, i) for j in range(2, NCH // 2) for i in range(sz)]
                )
            for j, i in order:
                nts = slice(i * P, (i + 1) * P)
                g, ii = pd_of[i]
                nc.tensor.matmul(
                    pds[g][:, ii, :],
                    z_sb[s][:, 2 * j : 2 * j + 2, nts],
                    w_sb[s][:, 2 * j : 2 * j + 2, :],
                    start=(j == 0),
                    stop=(j == NCH // 2 - 1),
                    perf_mode=DR,
                )
            for i, pd in enumerate(pds):
                width = len(groups[i])
                pi = pbase + i
                if 0 < s < len(sizes) - 1 and i == 0 and width == 2:
                    sv = dwork.tile([P, 2, ENC], F32, name="sv")
                    nc.vector.tensor_scalar(sv[:], pd[:], 1.0, None, op0=AX.mult)
                    sj = dwork.tile([P, 2, ENC], F32, name="sj")
                    nc.vector.scalar_tensor_tensor(
                        sj[:],
                        sv[:],
                        1.0,
                        sv[:],
                        op0=AX.mult,
                        op1=AX.mult,
                        accum_out=acc[:, pi : pi + 1],
                    )
                else:
                    sj = dwork.tile([P, width, ENC], F32, name="sj")
                    nc.scalar.activation(
                        sj[:],
                        pd[:, 0:width, :],
                        mybir.ActivationFunctionType.Square,
                        accum_out=acc[:, pi : pi + 1],
                    )
            pbase += len(pds)
        # flush all but the last segment's accumulators mid-phase; the final
        # piece issues from the scalar engine, which also ran the last
        # squares, so no cross-engine semaphore hop sits on the critical tail
        cut = npairs - len(seg_groups[-1])
        if cut > 0:
            nc.sync.dma_start(loss[:, 0:cut], acc[:, 0:cut])
        nc.scalar.dma_start(loss[:, cut:npairs], acc[:, cut:npairs])

    nc.finalize()
    return nc


_NC_CACHE = {}


def _get_nc(sizes=None):
    if sizes is None:
        key = next(reversed(_NC_CACHE))
    else:
        key = tuple(sizes)
        if key not in _NC_CACHE:
            _NC_CACHE[key] = build_nc(key)
    return _NC_CACHE[key]


def _route(X0, W_enc, C_w, C_b):
    # f64 router on host: argmax(X0 @ W_enc.T @ C_w.T + C_b) per row
    m = (C_w.astype(np.float64) @ W_enc.astype(np.float64)).T  # [OBS, K]
    logits = X0.astype(np.float64) @ m + C_b.astype(np.float64)
    return np.argmax(logits, axis=1)


def _pack(tiles_k):
    # Assign each (core, segment-slot) to one expert so every expert's tiles
    # fit in its segments. All cores share the same slot-size pattern so the
    # device program is SPMD-identical.
    total = sum(tiles_k)
    lo = max(1, -(-total // NCORES))
    hi = max(max(tiles_k), lo)
    for T in range(lo, hi + 1):
        sizes = _seg_sizes(T)
        slots = [
            (sz, core, si) for core in range(NCORES) for si, sz in enumerate(sizes)
        ]
        slots.sort()
        assign = {}
        ok = True
        for k in sorted(range(K), key=lambda k: -tiles_k[k]):
            need = tiles_k[k]
            while need > 0:
                if not slots:
                    ok = False
                    break
                # closest fit: smallest slot covering the need, else largest
                idx = next((i for i, s in enumerate(slots) if s[0] >= need), -1)
                sz, core, si = slots.pop(idx)
                assign[(core, si)] = k
                need -= sz
            if not ok:
                break
        if ok:
            return sizes, assign
    # unreachable: T = max(tiles_k) always fits (each expert gets one core)
    raise RuntimeError("packing failed")


def make_in_maps(X1, X0, U, W_enc, A_all, B_rest, C_w, C_b):
    inds = _route(X0, W_enc, C_w, C_b)
    tiles_k = [max(1, -(-int(c) // P)) for c in np.bincount(inds, minlength=K)]
    sizes, assign = _pack(tiles_k)

    wT = W_enc.T.astype(np.float32)  # [OBS, ENC]
    ae = wT[None] @ A_all.transpose(0, 2, 1).astype(np.float32)  # [K, OBS, ENC]
    b0 = np.eye(ENC, dtype=np.float32)[:ACT]
    ball = np.concatenate([b0[None], B_rest.astype(np.float32)], axis=0)
    wmat = np.zeros((K, NCH * P, ENC), dtype=np.float32)
    wmat[:, 0:OBS] = ae * WSCALE
    wmat[:, OBS : OBS + ACT] = ball * WSCALE
    wmat[:, OBS + P : OBS + P + OBS] = -wT[None] * WSCALE
    wq = np.asarray(wmat, dtype=NP_FP8).reshape(K, NCH, P, ENC).transpose(0, 2, 1, 3)
    wq = np.ascontiguousarray(wq)  # [K, P, NCH, ENC]
    wzero = np.zeros((P, NCH, ENC), dtype=NP_FP8)

    rows_k = [np.nonzero(inds == k)[0] for k in range(K)]
    used_k = [0] * K

    in_maps = [{} for _ in range(NCORES)]
    for core in range(NCORES):
        for si, sz in enumerate(sizes):
            cap = sz * P
            k = assign.get((core, si))
            Z = np.zeros((ZCH * P, cap), dtype=np.float32)
            if k is not None:
                rk = rows_k[k][used_k[k] : used_k[k] + cap]
                used_k[k] += len(rk)
                c = len(rk)
                Z[0:OBS, :c] = X0[rk].T
                Z[OBS : OBS + ACT, :c] = U[rk].T
                Z[OBS + P : OBS + P + OBS, :c] = X1[rk].T
            zq = np.asarray(Z, dtype=NP_FP8).reshape(ZCH, P, cap)
            in_maps[core][f"z{si}"] = np.ascontiguousarray(zq.transpose(1, 0, 2))
            in_maps[core][f"w{si}"] = wq[k] if k is not None else wzero
    assert all(used_k[k] == len(rows_k[k]) for k in range(K)), "packing dropped rows"
    return in_maps, sizes


def kernel(X1, X0, U, W_enc, A_all, B_rest, C_w, C_b):
    in_maps, sizes = make_in_maps(X1, X0, U, W_enc, A_all, B_rest, C_w, C_b)
    nc = _get_nc(sizes)
    res = bass_utils.run_bass_kernel_spmd(nc, in_maps, list(range(NCORES)))
    total = sum(float(r["loss_out"].sum()) for r in res.results)
    return np.float32(ALPHA * total / (WSCALE * WSCALE * ENC * N))
